# revision 42
# baseline (speedup 1.0000x reference)
"""SnapKV sparse attention on 8 Trainium2 NeuronCores.

Sharding: core = (seq b = core//2, head-half = core%2). Each core handles one
sequence's 16 query heads (4 kv heads): full causal prefill attention in an
S^T layout (fp32r matmuls), plus an exact-fp32 SnapKV importance partial for
its own heads. Host sums the two per-sequence importance partials (the
"all-reduce" over head shards), does the top-k selection, and scatters the
kept k/v rows into the caches at the slot_mapping slots.

Self-contained: shapes hardcoded for the nn_Attention problem
(q [4096,32,128], k/v [4096,8,128], caches [8192,1024]).
"""
import os

import numpy as np

B, L = 4, 1024
H, HKV, DH = 32, 8, 128
HPC = 16       # q heads per core
KVPC = 4       # kv heads per core
SQ, LIM = 128, 512
SCALE = float(DH) ** -0.5
NEG = -1e30
N_CORES = 8
NT = L // 128  # 8 row tiles per sequence

TRACE = False
LAST_RESULTS = {}

_prog = None


def _build_program(cfg=None):
    import concourse.tile as tile
    from concourse import bacc, mybir

    cfg = dict(cfg or {})
    nS = cfg.get("nS", 3)        # QK S^T psum tiles
    nT = cfg.get("nT", 3)        # transpose/SL psum tiles
    nOT = cfg.get("nOT", 1)      # O^T accumulators
    nDen = cfg.get("nDen", 1)    # denominator accumulators
    nQ = cfg.get("nQ", 2)        # q-side sbuf lookahead (heads in flight)
    nP = cfg.get("nP", 6)        # P sbuf tiles

    F32 = mybir.dt.float32
    F32R = mybir.dt.float32r
    AF = mybir.ActivationFunctionType
    ALU = mybir.AluOpType

    nc = bacc.Bacc("TRN2", target_bir_lowering=False, debug=False,
                   enable_asserts=False, num_devices=N_CORES)

    q_d = nc.dram_tensor("q_in", [L, HPC, DH], F32, kind="ExternalInput").ap()
    k_d = nc.dram_tensor("k_in", [L, KVPC, DH], F32, kind="ExternalInput").ap()
    v_d = nc.dram_tensor("v_in", [L, KVPC, DH], F32, kind="ExternalInput").ap()
    o_d = nc.dram_tensor("o_out", [L, HPC, DH], F32, kind="ExternalOutput").ap()
    imp_d = nc.dram_tensor("imp_out", [128, L // 128], F32, kind="ExternalOutput").ap()

    with tile.TileContext(nc) as tc:
        with (tc.tile_pool(name="const", bufs=1) as constp,
              tc.tile_pool(name="kv", bufs=1) as kvp,
              tc.tile_pool(name="knat", bufs=2) as knatp,
              tc.tile_pool(name="qload", bufs=nQ) as qloadp,
              tc.tile_pool(name="qt", bufs=nQ) as qtp,
              tc.tile_pool(name="p", bufs=nP) as pp,
              tc.tile_pool(name="onorm", bufs=4) as onp_,
              tc.tile_pool(name="osb", bufs=2) as osbp,
              tc.tile_pool(name="psS", bufs=nS, space="PSUM") as psS,
              tc.tile_pool(name="psT", bufs=nT, space="PSUM") as psT,
              tc.tile_pool(name="psOT", bufs=nOT, space="PSUM") as psOT,
              tc.tile_pool(name="psDen", bufs=nDen, space="PSUM") as psDen):

            # ---- constants ----
            ident = constp.tile([128, 128], F32)
            nc.gpsimd.memset(ident[:], 0.0)
            nc.gpsimd.affine_select(out=ident[:], in_=ident[:],
                                    compare_op=ALU.not_equal, fill=1.0,
                                    base=0, pattern=[[-1, 128]],
                                    channel_multiplier=1)
            # additive causal mask for S diag tiles [q part, k free]:
            # keep where k <= q, else NEG
            maskL = constp.tile([128, 128], F32)
            nc.gpsimd.memset(maskL[:], 0.0)
            nc.gpsimd.affine_select(out=maskL[:], in_=maskL[:],
                                    compare_op=ALU.is_ge, fill=NEG,
                                    base=0, pattern=[[-1, 128]],
                                    channel_multiplier=1)
            ident32r = constp.tile([128, 128], F32R)
            nc.vector.tensor_copy(ident32r[:], ident[:])
            ones32 = constp.tile([128, 128], F32)
            nc.gpsimd.memset(ones32[:], 1.0)
            ones_r = constp.tile([128, 128], F32R)
            nc.vector.tensor_copy(ones_r[:], ones32[:])

            # importance accumulator in SBUF (per-head PSUM partials are
            # DVE-added into it): imp_acc[k_local, m] = importance of key
            # position m*128 + k_local
            imp_acc = constp.tile([128, NT], F32)
            nc.gpsimd.memset(imp_acc[:], 0.0)

            # ---- load + transpose K (fp32r + fp32), load V (fp32r) ----
            kt_r, kt_32, v_r = [], [], []
            for hv in range(KVPC):
                knat = knatp.tile([128, NT, 128], F32, tag="knat")
                nc.sync.dma_start(
                    knat[:], k_d[:, hv, :].rearrange("(j p) d -> p j d", p=128))
                vt = kvp.tile([128, NT, 128], F32R, tag=f"v{hv}")
                nc.gpsimd.dma_start(
                    vt[:], v_d[:, hv, :].rearrange("(j p) d -> p j d", p=128))
                ktr = kvp.tile([128, L], F32R, tag=f"ktr{hv}")
                kt32 = kvp.tile([128, L], F32, tag=f"kt32{hv}")
                for j in range(NT):
                    tp = psT.tile([128, 512], F32, tag="t")
                    nc.tensor.transpose(tp[:, 0:128], knat[:, j, :], ident[:])
                    # kt32 = exact fp32 SCALE*K^T (for the fp32 importance
                    # matmuls); ktr = its fp32r rounding (main path)
                    nc.vector.tensor_scalar_mul(kt32[:, j * 128:(j + 1) * 128],
                                                tp[:, 0:128], SCALE)
                    nc.vector.tensor_copy(ktr[:, j * 128:(j + 1) * 128],
                                          kt32[:, j * 128:(j + 1) * 128])
                kt_r.append(ktr)
                kt_32.append(kt32)
                v_r.append(vt)

            # ---- per q-head pipeline, software-pipelined ----
            # Tail work of head h (O transposes + store) is emitted after head
            # h+1's matmul stream so the in-order PE never stalls on the DVE
            # normalization chain.
            pending = None  # (h, [otn chunk0, otn chunk1])

            def emit_tail(pend):
                ph, otns = pend
                osb = osbp.tile([128, NT, 128], F32, tag="osb", name=f"osb{ph}")
                for c in range(2):
                    for t in range(4):
                        o_ps = psT.tile([128, 512], F32R, tag="t",
                                        name=f"ops{ph}_{c}_{t}")
                        nc.tensor.transpose(
                            o_ps[:, 0:128],
                            otns[c][:, t * 128:(t + 1) * 128], ident32r[:])
                        nc.any.tensor_copy(osb[:, c * 4 + t, :], o_ps[:, 0:128])
                nc.sync.dma_start(
                    o_d[:, ph, :].rearrange("(j p) d -> p j d", p=128), osb[:])

            for h in range(HPC):
                hv = h // 4
                qnat32 = qloadp.tile([128, 128], F32, tag="qnat32")
                nc.sync.dma_start(qnat32[:], q_d[(NT - 1) * 128:L, h, :])
                qnat = qloadp.tile([128, NT, 128], F32R, tag="qnat")
                nc.gpsimd.dma_start(
                    qnat[:], q_d[:, h, :].rearrange("(j p) d -> p j d", p=128))
                qt = qtp.tile([128, L], F32R, tag="qt")       # Q^T, fp32r
                qtl32 = qtp.tile([128, 128], F32, tag="qtl")  # last tile, fp32
                for j in range(NT):
                    tp = psT.tile([128, 512], F32R, tag="t")
                    nc.tensor.transpose(tp[:, 0:128], qnat[:, j, :], ident32r[:])
                    nc.vector.tensor_copy(qt[:, j * 128:(j + 1) * 128],
                                          tp[:, 0:128])
                tp32 = psT.tile([128, 512], F32, tag="t", name=f"tpq32_{h}")
                nc.tensor.transpose(tp32[:, 0:128], qnat32[:], ident[:])
                nc.vector.tensor_copy(qtl32[:], tp32[:, 0:128])

                otns = []
                for c in range(2):
                    q0 = c * 512
                    ot_ps = psOT.tile([128, 512], F32, tag="ot")
                    den_ps = psDen.tile([128, 512], F32, tag="den")
                    jmax = 4 * c + 3

                    def emit_qk_exp(j):
                        off = max(0, j * 128 - q0)
                        s_ps = psS.tile([128, 512], F32, tag="s", name=f"s{h}_{c}_{j}")
                        nc.tensor.matmul(s_ps[:, off:512],
                                         kt_r[hv][:, j * 128:(j + 1) * 128],
                                         qt[:, q0 + off:q0 + 512],
                                         start=True, stop=True)
                        p_t = pp.tile([128, 512], F32R, tag="p", name=f"p{h}_{c}_{j}")
                        nc.scalar.activation(p_t[:, off:512], s_ps[:, off:512],
                                             AF.Exp)
                        if j * 128 >= q0:
                            # zero the acausal (k > q) half of the diagonal
                            # subtile, on the otherwise-idle gpsimd engine
                            nc.gpsimd.affine_select(
                                out=p_t[:, off:off + 128],
                                in_=p_t[:, off:off + 128],
                                compare_op=ALU.is_ge, fill=0.0,
                                base=0, pattern=[[1, 128]],
                                channel_multiplier=-1)
                        return off, p_t

                    def emit_pv_den(j, off, p_t):
                        nc.tensor.matmul(ot_ps[:, off:512], v_r[hv][:, j, :],
                                         p_t[:, off:512],
                                         start=(j == 0), stop=(j == jmax))
                        nc.tensor.matmul(den_ps[:, off:512], ones_r[:],
                                         p_t[:, off:512],
                                         start=(j == 0), stop=(j == jmax))

                    # skew by SKEW j's so PV_j never heads the PE queue
                    # before exp_j is done
                    SKEW = 2
                    queue = []
                    for j in range(jmax + 1):
                        off, p_t = emit_qk_exp(j)
                        queue.append((j, off, p_t))
                        if len(queue) > SKEW:
                            emit_pv_den(*queue.pop(0))
                    for item in queue:
                        emit_pv_den(*item)

                    rbc = onp_.tile([128, 512], F32, tag="r", name=f"rbc{h}_{c}")
                    nc.vector.reciprocal(rbc[:], den_ps[:])
                    otn = onp_.tile([128, 512], F32R, tag="otn", name=f"otn{h}_{c}")
                    nc.vector.tensor_mul(otn[:], ot_ps[:], rbc[:])
                    otns.append(otn)

                # ---- exact fp32 SnapKV importance for this head ----
                pl = pp.tile([128, L], F32, tag="pl", name=f"pl{h}")
                for half in range(2):
                    sl_ps = psT.tile([128, 512], F32, tag="t",
                                     name=f"sl{h}_{half}")
                    nc.tensor.matmul(sl_ps[:], qtl32[:],
                                     kt_32[hv][:, half * 512:(half + 1) * 512],
                                     start=True, stop=True)
                    if half == 1:
                        nc.vector.tensor_add(sl_ps[:, 384:512],
                                             sl_ps[:, 384:512], maskL[:])
                    nc.scalar.activation(pl[:, half * 512:(half + 1) * 512],
                                         sl_ps[:], AF.Exp)
                denl = pp.tile([128, 1], F32, tag="denl", name=f"denl{h}")
                nc.vector.tensor_reduce(denl[:], pl[:],
                                        axis=mybir.AxisListType.X,
                                        op=mybir.AluOpType.add)
                rl = pp.tile([128, 1], F32, tag="rl", name=f"rl{h}")
                nc.vector.reciprocal(rl[:], denl[:])
                imp_h = psOT.tile([128, 512], F32, tag="ot", name=f"imph{h}")
                for m in range(NT):
                    nc.tensor.matmul(imp_h[:, m:m + 1],
                                     pl[:, m * 128:(m + 1) * 128], rl[:],
                                     start=(m == 0), stop=(m == NT - 1))
                nc.vector.tensor_add(imp_acc[:], imp_acc[:], imp_h[:, 0:NT])

                if pending is not None:
                    emit_tail(pending)
                pending = (h, otns)

            emit_tail(pending)
            nc.sync.dma_start(imp_d[:], imp_acc[:])

    nc.compile()
    return nc


def _get_program():
    global _prog
    if _prog is None:
        _prog = _build_program()
    return _prog


class _SubprocResults:
    def __init__(self, results):
        self.results = results
        self.exec_time_ns = None
        self.mean_exec_time_ns = None


def _run_spmd_subprocess(in_maps):
    """Fallback: run the SPMD launch in a fresh process. The axon device
    occasionally wedges (NRT_EXEC_UNIT_UNRECOVERABLE) and the wedged state
    can stick to the failing process; a fresh process recovers."""
    import pickle
    import subprocess
    import sys
    import tempfile

    with tempfile.TemporaryDirectory() as td:
        with open(os.path.join(td, "in.pkl"), "wb") as f:
            pickle.dump(in_maps, f)
        code = (
            "import pickle, sys\n"
            f"sys.path.insert(0, {os.path.dirname(os.path.abspath(__file__))!r})\n"
            "import kernel as kmod\n"
            f"in_maps = pickle.load(open({os.path.join(td, 'in.pkl')!r}, 'rb'))\n"
            "from concourse import bass_utils\n"
            "res = bass_utils.run_bass_kernel_spmd(\n"
            "    kmod._get_program(), in_maps,\n"
            "    core_ids=list(range(kmod.N_CORES)))\n"
            f"pickle.dump(res.results, open({os.path.join(td, 'out.pkl')!r}, 'wb'))\n"
        )
        subprocess.run([sys.executable, "-c", code], check=True, timeout=1200)
        with open(os.path.join(td, "out.pkl"), "rb") as f:
            return _SubprocResults(pickle.load(f))


def _run_spmd(nc, in_maps, trace):
    from concourse import bass_utils
    last_err = None
    for attempt in range(3):
        try:
            return bass_utils.run_bass_kernel_spmd(
                nc, in_maps, core_ids=list(range(N_CORES)), trace=trace)
        except Exception as e:  # transient NRT_EXEC_UNIT_UNRECOVERABLE wedges
            last_err = e
            import time
            time.sleep(2.0 * (attempt + 1))
    try:
        return _run_spmd_subprocess(in_maps)
    except Exception:
        raise last_err


def kernel(q, k, v, k_cache, v_cache, slot_mapping, cu_seqlens):
    q = np.ascontiguousarray(q, dtype=np.float32)
    k = np.ascontiguousarray(k, dtype=np.float32)
    v = np.ascontiguousarray(v, dtype=np.float32)
    nc = _get_program()

    in_maps = []
    for core in range(N_CORES):
        b, half = core // 2, core % 2
        in_maps.append({
            "q_in": np.ascontiguousarray(
                q[b * L:(b + 1) * L, half * HPC:(half + 1) * HPC, :]),
            "k_in": np.ascontiguousarray(
                k[b * L:(b + 1) * L, half * KVPC:(half + 1) * KVPC, :]),
            "v_in": np.ascontiguousarray(
                v[b * L:(b + 1) * L, half * KVPC:(half + 1) * KVPC, :]),
        })

    import time as _time
    _t0 = _time.perf_counter()
    res = _run_spmd(nc, in_maps, TRACE)
    _t1 = _time.perf_counter()
    LAST_RESULTS["exec_time_ns"] = res.exec_time_ns
    LAST_RESULTS["mean_exec_time_ns"] = res.mean_exec_time_ns
    LAST_RESULTS["wall_run_ns"] = int((_t1 - _t0) * 1e9)

    o = np.empty((B * L, H, DH), np.float32)
    imp = np.zeros((B, L), np.float64)
    for core in range(N_CORES):
        b, half = core // 2, core % 2
        o[b * L:(b + 1) * L, half * HPC:(half + 1) * HPC, :] = \
            res.results[core]["o_out"]
        imp[b] += res.results[core]["imp_out"].T.reshape(-1).astype(np.float64)
    imp = imp.astype(np.float32)

    # top-k selection (matches jax.lax.top_k tie-breaking: lower index wins)
    keep = np.stack([np.sort(np.argsort(-imp[b], kind="stable")[:LIM])
                     for b in range(B)])  # [B, LIM] sorted kept positions

    kb = k.reshape(B, L, HKV, DH)
    vb = v.reshape(B, L, HKV, DH)
    kept_k = np.take_along_axis(kb, keep[:, :, None, None], axis=1)
    kept_v = np.take_along_axis(vb, keep[:, :, None, None], axis=1)

    slot_mapping = np.asarray(slot_mapping)
    slots = slot_mapping.reshape(B, L)[:, :LIM].reshape(-1)
    kc = np.array(k_cache, dtype=np.float32, copy=True)
    vc = np.array(v_cache, dtype=np.float32, copy=True)
    kc[slots] = kept_k.reshape(-1, HKV * DH)
    vc[slots] = kept_v.reshape(-1, HKV * DH)
    return o, kc, vc


# revision 45
# speedup vs baseline: 1.0149x; 1.0149x over previous
"""SnapKV sparse attention on 8 Trainium2 NeuronCores.

Sharding: core = (seq b = core//2, head-half = core%2). Each core handles one
sequence's 16 query heads (4 kv heads): full causal prefill attention in an
S^T layout (fp32r matmuls), plus an exact-fp32 SnapKV importance partial for
its own heads. Host sums the two per-sequence importance partials (the
"all-reduce" over head shards), does the top-k selection, and scatters the
kept k/v rows into the caches at the slot_mapping slots.

Self-contained: shapes hardcoded for the nn_Attention problem
(q [4096,32,128], k/v [4096,8,128], caches [8192,1024]).
"""
import os

import numpy as np

B, L = 4, 1024
H, HKV, DH = 32, 8, 128
HPC = 16       # q heads per core
KVPC = 4       # kv heads per core
SQ, LIM = 128, 512
SCALE = float(DH) ** -0.5
NEG = -1e30
N_CORES = 8
NT = L // 128  # 8 row tiles per sequence

TRACE = False
LAST_RESULTS = {}

_prog = None


def _build_program(cfg=None):
    import concourse.tile as tile
    from concourse import bacc, mybir

    cfg = dict(cfg or {})
    nS = cfg.get("nS", 3)        # QK S^T psum tiles
    nT = cfg.get("nT", 3)        # transpose/SL psum tiles
    nOT = cfg.get("nOT", 1)      # O^T accumulators
    nDen = cfg.get("nDen", 1)    # denominator accumulators
    nQ = cfg.get("nQ", 2)        # q-side sbuf lookahead (heads in flight)
    nP = cfg.get("nP", 6)        # P sbuf tiles

    F32 = mybir.dt.float32
    F32R = mybir.dt.float32r
    AF = mybir.ActivationFunctionType
    ALU = mybir.AluOpType

    nc = bacc.Bacc("TRN2", target_bir_lowering=False, debug=False,
                   enable_asserts=False, num_devices=N_CORES)

    q_d = nc.dram_tensor("q_in", [L, HPC, DH], F32, kind="ExternalInput").ap()
    k_d = nc.dram_tensor("k_in", [L, KVPC, DH], F32, kind="ExternalInput").ap()
    v_d = nc.dram_tensor("v_in", [L, KVPC, DH], F32, kind="ExternalInput").ap()
    o_d = nc.dram_tensor("o_out", [L, HPC, DH], F32, kind="ExternalOutput").ap()
    imp_d = nc.dram_tensor("imp_out", [128, L // 128], F32, kind="ExternalOutput").ap()

    with tile.TileContext(nc) as tc:
        with (tc.tile_pool(name="const", bufs=1) as constp,
              tc.tile_pool(name="kv", bufs=1) as kvp,
              tc.tile_pool(name="knat", bufs=2) as knatp,
              tc.tile_pool(name="qload", bufs=nQ) as qloadp,
              tc.tile_pool(name="qt", bufs=nQ) as qtp,
              tc.tile_pool(name="p", bufs=nP) as pp,
              tc.tile_pool(name="onorm", bufs=4) as onp_,
              tc.tile_pool(name="osb", bufs=2) as osbp,
              tc.tile_pool(name="psS", bufs=nS, space="PSUM") as psS,
              tc.tile_pool(name="psT", bufs=nT, space="PSUM") as psT,
              tc.tile_pool(name="psOT", bufs=nOT, space="PSUM") as psOT,
              tc.tile_pool(name="psDen", bufs=nDen, space="PSUM") as psDen):

            # ---- constants ----
            ident = constp.tile([128, 128], F32)
            nc.gpsimd.memset(ident[:], 0.0)
            nc.gpsimd.affine_select(out=ident[:], in_=ident[:],
                                    compare_op=ALU.not_equal, fill=1.0,
                                    base=0, pattern=[[-1, 128]],
                                    channel_multiplier=1)
            # additive causal mask for S diag tiles [q part, k free]:
            # keep where k <= q, else NEG
            maskL = constp.tile([128, 128], F32)
            nc.gpsimd.memset(maskL[:], 0.0)
            nc.gpsimd.affine_select(out=maskL[:], in_=maskL[:],
                                    compare_op=ALU.is_ge, fill=NEG,
                                    base=0, pattern=[[-1, 128]],
                                    channel_multiplier=1)
            ident32r = constp.tile([128, 128], F32R)
            nc.vector.tensor_copy(ident32r[:], ident[:])
            ones32 = constp.tile([128, 128], F32)
            nc.gpsimd.memset(ones32[:], 1.0)
            ones_r = constp.tile([128, 128], F32R)
            nc.vector.tensor_copy(ones_r[:], ones32[:])

            # importance accumulator in SBUF (per-head PSUM partials are
            # DVE-added into it): imp_acc[k_local, m] = importance of key
            # position m*128 + k_local
            imp_acc = constp.tile([128, NT], F32)
            nc.gpsimd.memset(imp_acc[:], 0.0)

            # ---- load + transpose K (fp32r + fp32), load V (fp32r) ----
            # issue all K/V DMAs first so the transposes stream behind them
            kt_r, kt_32, v_r, _knats = [], [], [], []
            for hv in range(KVPC):
                knat = knatp.tile([128, NT, 128], F32, tag=f"knat{hv}")
                nc.sync.dma_start(
                    knat[:], k_d[:, hv, :].rearrange("(j p) d -> p j d", p=128))
                _knats.append(knat)
                vt = kvp.tile([128, NT, 128], F32R, tag=f"v{hv}")
                nc.gpsimd.dma_start(
                    vt[:], v_d[:, hv, :].rearrange("(j p) d -> p j d", p=128))
                v_r.append(vt)
            for hv in range(KVPC):
                knat = _knats[hv]
                ktr = kvp.tile([128, L], F32R, tag=f"ktr{hv}")
                kt32 = kvp.tile([128, L], F32, tag=f"kt32{hv}")
                for j in range(NT):
                    tp = psT.tile([128, 512], F32, tag="t")
                    nc.tensor.transpose(tp[:, 0:128], knat[:, j, :], ident[:])
                    # kt32 = exact fp32 SCALE*K^T (for the fp32 importance
                    # matmuls); ktr = its fp32r rounding (main path)
                    nc.vector.tensor_scalar_mul(kt32[:, j * 128:(j + 1) * 128],
                                                tp[:, 0:128], SCALE)
                    nc.vector.tensor_copy(ktr[:, j * 128:(j + 1) * 128],
                                          kt32[:, j * 128:(j + 1) * 128])
                kt_r.append(ktr)
                kt_32.append(kt32)

            # ---- per q-head pipeline, software-pipelined ----
            # Tail work of head h (O transposes + store) is emitted after head
            # h+1's matmul stream so the in-order PE never stalls on the DVE
            # normalization chain.
            pending = None  # (h, [otn chunk0, otn chunk1])

            def emit_tail(pend):
                ph, otns = pend
                osb = osbp.tile([128, NT, 128], F32, tag="osb", name=f"osb{ph}")
                for c in range(2):
                    for t in range(4):
                        o_ps = psT.tile([128, 512], F32R, tag="t",
                                        name=f"ops{ph}_{c}_{t}")
                        nc.tensor.transpose(
                            o_ps[:, 0:128],
                            otns[c][:, t * 128:(t + 1) * 128], ident32r[:])
                        nc.any.tensor_copy(osb[:, c * 4 + t, :], o_ps[:, 0:128])
                nc.sync.dma_start(
                    o_d[:, ph, :].rearrange("(j p) d -> p j d", p=128), osb[:])

            for h in range(HPC):
                hv = h // 4
                qnat32 = qloadp.tile([128, 128], F32, tag="qnat32")
                nc.sync.dma_start(qnat32[:], q_d[(NT - 1) * 128:L, h, :])
                qnat = qloadp.tile([128, NT, 128], F32R, tag="qnat")
                nc.gpsimd.dma_start(
                    qnat[:], q_d[:, h, :].rearrange("(j p) d -> p j d", p=128))
                qt = qtp.tile([128, L], F32R, tag="qt")       # Q^T, fp32r
                qtl32 = qtp.tile([128, 128], F32, tag="qtl")  # last tile, fp32
                for j in range(NT):
                    tp = psT.tile([128, 512], F32R, tag="t")
                    nc.tensor.transpose(tp[:, 0:128], qnat[:, j, :], ident32r[:])
                    nc.vector.tensor_copy(qt[:, j * 128:(j + 1) * 128],
                                          tp[:, 0:128])
                tp32 = psT.tile([128, 512], F32, tag="t", name=f"tpq32_{h}")
                nc.tensor.transpose(tp32[:, 0:128], qnat32[:], ident[:])
                nc.vector.tensor_copy(qtl32[:], tp32[:, 0:128])

                otns = []
                for c in range(2):
                    q0 = c * 512
                    ot_ps = psOT.tile([128, 512], F32, tag="ot")
                    den_ps = psDen.tile([128, 512], F32, tag="den")
                    jmax = 4 * c + 3

                    def emit_qk_exp(j):
                        off = max(0, j * 128 - q0)
                        s_ps = psS.tile([128, 512], F32, tag="s", name=f"s{h}_{c}_{j}")
                        nc.tensor.matmul(s_ps[:, off:512],
                                         kt_r[hv][:, j * 128:(j + 1) * 128],
                                         qt[:, q0 + off:q0 + 512],
                                         start=True, stop=True)
                        p_t = pp.tile([128, 512], F32R, tag="p", name=f"p{h}_{c}_{j}")
                        nc.scalar.activation(p_t[:, off:512], s_ps[:, off:512],
                                             AF.Exp)
                        if j * 128 >= q0:
                            # zero the acausal (k > q) half of the diagonal
                            # subtile, on the otherwise-idle gpsimd engine
                            nc.gpsimd.affine_select(
                                out=p_t[:, off:off + 128],
                                in_=p_t[:, off:off + 128],
                                compare_op=ALU.is_ge, fill=0.0,
                                base=0, pattern=[[1, 128]],
                                channel_multiplier=-1)
                        return off, p_t

                    def emit_pv_den(j, off, p_t):
                        nc.tensor.matmul(ot_ps[:, off:512], v_r[hv][:, j, :],
                                         p_t[:, off:512],
                                         start=(j == 0), stop=(j == jmax))
                        nc.tensor.matmul(den_ps[:, off:512], ones_r[:],
                                         p_t[:, off:512],
                                         start=(j == 0), stop=(j == jmax))

                    # skew by SKEW j's so PV_j never heads the PE queue
                    # before exp_j is done
                    SKEW = 2
                    queue = []
                    for j in range(jmax + 1):
                        off, p_t = emit_qk_exp(j)
                        queue.append((j, off, p_t))
                        if len(queue) > SKEW:
                            emit_pv_den(*queue.pop(0))
                    for item in queue:
                        emit_pv_den(*item)

                    rbc = onp_.tile([128, 512], F32, tag="r", name=f"rbc{h}_{c}")
                    nc.vector.reciprocal(rbc[:], den_ps[:])
                    otn = onp_.tile([128, 512], F32R, tag="otn", name=f"otn{h}_{c}")
                    nc.vector.tensor_mul(otn[:], ot_ps[:], rbc[:])
                    otns.append(otn)

                # ---- exact fp32 SnapKV importance for this head ----
                pl = pp.tile([128, L], F32, tag="pl", name=f"pl{h}")
                for half in range(2):
                    sl_ps = psT.tile([128, 512], F32, tag="t",
                                     name=f"sl{h}_{half}")
                    nc.tensor.matmul(sl_ps[:], qtl32[:],
                                     kt_32[hv][:, half * 512:(half + 1) * 512],
                                     start=True, stop=True)
                    if half == 1:
                        nc.vector.tensor_add(sl_ps[:, 384:512],
                                             sl_ps[:, 384:512], maskL[:])
                    nc.scalar.activation(pl[:, half * 512:(half + 1) * 512],
                                         sl_ps[:], AF.Exp)
                denl = pp.tile([128, 1], F32, tag="denl", name=f"denl{h}")
                nc.vector.tensor_reduce(denl[:], pl[:],
                                        axis=mybir.AxisListType.X,
                                        op=mybir.AluOpType.add)
                rl = pp.tile([128, 1], F32, tag="rl", name=f"rl{h}")
                nc.vector.reciprocal(rl[:], denl[:])
                imp_h = psOT.tile([128, 512], F32, tag="ot", name=f"imph{h}")
                for m in range(NT):
                    nc.tensor.matmul(imp_h[:, m:m + 1],
                                     pl[:, m * 128:(m + 1) * 128], rl[:],
                                     start=(m == 0), stop=(m == NT - 1))
                nc.vector.tensor_add(imp_acc[:], imp_acc[:], imp_h[:, 0:NT])

                if pending is not None:
                    emit_tail(pending)
                pending = (h, otns)

            emit_tail(pending)
            nc.sync.dma_start(imp_d[:], imp_acc[:])

    nc.compile()
    return nc


def _get_program():
    global _prog
    if _prog is None:
        _prog = _build_program()
    return _prog


class _SubprocResults:
    def __init__(self, results):
        self.results = results
        self.exec_time_ns = None
        self.mean_exec_time_ns = None


def _run_spmd_subprocess(in_maps):
    """Fallback: run the SPMD launch in a fresh process. The axon device
    occasionally wedges (NRT_EXEC_UNIT_UNRECOVERABLE) and the wedged state
    can stick to the failing process; a fresh process recovers."""
    import pickle
    import subprocess
    import sys
    import tempfile

    with tempfile.TemporaryDirectory() as td:
        with open(os.path.join(td, "in.pkl"), "wb") as f:
            pickle.dump(in_maps, f)
        code = (
            "import pickle, sys\n"
            f"sys.path.insert(0, {os.path.dirname(os.path.abspath(__file__))!r})\n"
            "import kernel as kmod\n"
            f"in_maps = pickle.load(open({os.path.join(td, 'in.pkl')!r}, 'rb'))\n"
            "from concourse import bass_utils\n"
            "res = bass_utils.run_bass_kernel_spmd(\n"
            "    kmod._get_program(), in_maps,\n"
            "    core_ids=list(range(kmod.N_CORES)))\n"
            f"pickle.dump(res.results, open({os.path.join(td, 'out.pkl')!r}, 'wb'))\n"
        )
        subprocess.run([sys.executable, "-c", code], check=True, timeout=1200)
        with open(os.path.join(td, "out.pkl"), "rb") as f:
            return _SubprocResults(pickle.load(f))


def _run_spmd(nc, in_maps, trace):
    from concourse import bass_utils
    last_err = None
    for attempt in range(3):
        try:
            return bass_utils.run_bass_kernel_spmd(
                nc, in_maps, core_ids=list(range(N_CORES)), trace=trace)
        except Exception as e:  # transient NRT_EXEC_UNIT_UNRECOVERABLE wedges
            last_err = e
            import time
            time.sleep(2.0 * (attempt + 1))
    try:
        return _run_spmd_subprocess(in_maps)
    except Exception:
        raise last_err


def kernel(q, k, v, k_cache, v_cache, slot_mapping, cu_seqlens):
    q = np.ascontiguousarray(q, dtype=np.float32)
    k = np.ascontiguousarray(k, dtype=np.float32)
    v = np.ascontiguousarray(v, dtype=np.float32)
    nc = _get_program()

    in_maps = []
    for core in range(N_CORES):
        b, half = core // 2, core % 2
        in_maps.append({
            "q_in": np.ascontiguousarray(
                q[b * L:(b + 1) * L, half * HPC:(half + 1) * HPC, :]),
            "k_in": np.ascontiguousarray(
                k[b * L:(b + 1) * L, half * KVPC:(half + 1) * KVPC, :]),
            "v_in": np.ascontiguousarray(
                v[b * L:(b + 1) * L, half * KVPC:(half + 1) * KVPC, :]),
        })

    import time as _time
    _t0 = _time.perf_counter()
    res = _run_spmd(nc, in_maps, TRACE)
    _t1 = _time.perf_counter()
    LAST_RESULTS["exec_time_ns"] = res.exec_time_ns
    LAST_RESULTS["mean_exec_time_ns"] = res.mean_exec_time_ns
    LAST_RESULTS["wall_run_ns"] = int((_t1 - _t0) * 1e9)

    o = np.empty((B * L, H, DH), np.float32)
    imp = np.zeros((B, L), np.float64)
    for core in range(N_CORES):
        b, half = core // 2, core % 2
        o[b * L:(b + 1) * L, half * HPC:(half + 1) * HPC, :] = \
            res.results[core]["o_out"]
        imp[b] += res.results[core]["imp_out"].T.reshape(-1).astype(np.float64)
    imp = imp.astype(np.float32)

    # top-k selection (matches jax.lax.top_k tie-breaking: lower index wins)
    keep = np.stack([np.sort(np.argsort(-imp[b], kind="stable")[:LIM])
                     for b in range(B)])  # [B, LIM] sorted kept positions

    kb = k.reshape(B, L, HKV, DH)
    vb = v.reshape(B, L, HKV, DH)
    kept_k = np.take_along_axis(kb, keep[:, :, None, None], axis=1)
    kept_v = np.take_along_axis(vb, keep[:, :, None, None], axis=1)

    slot_mapping = np.asarray(slot_mapping)
    slots = slot_mapping.reshape(B, L)[:, :LIM].reshape(-1)
    kc = np.array(k_cache, dtype=np.float32, copy=True)
    vc = np.array(v_cache, dtype=np.float32, copy=True)
    kc[slots] = kept_k.reshape(-1, HKV * DH)
    vc[slots] = kept_v.reshape(-1, HKV * DH)
    return o, kc, vc
